# revision 34
# baseline (speedup 1.0000x reference)
"""Trainium2 8-core Bass kernel for nn_Decoder_Layer_37177236914647.

Decoder layer: self-MHA(+causal mask) -> +res -> LN -> cross-MHA -> +res -> LN
-> FFN(2x dense, no act) -> +res -> LN.  Softmax is over the BATCH axis
(axis=0), faithful to the original model: w[b,h,q,k] = exp(s_b)/sum_b' exp(s_b').
With the reference's fp32 "+ mask*-1e9" the masked positions collapse to
exactly 0.25 for every batch (|scores| << ulp(1e9)=64), reproduced here with a
blend E' = E*(1-m) + m before the batch normalization.

Sharding: attention is head-parallel (16 heads / 8 cores = 2 heads per core;
the batch softmax is local per head); LayerNorms and the FFN are
token-parallel (512 tokens per core).  Since the harness hands every core the
FULL inputs, projections off the inputs (Q1/K1/V1 from dec, K2/V2 from enc)
need no communication at all.  The only cross-core traffic is three bf16
AllToAlls of 1MB output each (vs AllReduce+2 AllGathers+fp32 AllToAll in a
pure head-sharded design):
  A2A#1  attn1    feature-shard -> token-shard   (LN1 becomes local)
  A2A#2  Q2       token-shard  -> head-shard     (for head-sharded MHA2)
  A2A#3  attn2    feature-shard -> token-shard   (LN2/FFN/LN3 local)
LN1/LN2 are folded into the downstream projections: Q2 = (Wq2^T x1 - mu (x)
colsum(Wq2)) * r and h = (W1^T x2 - mu2 (x) colsum(W1)) * r2, so the
normalize step never materializes on the critical path.
Output returned token-sharded, reassembled and transposed on host.
"""
import numpy as np
import ml_dtypes

import concourse.bass as bass
import concourse.mybir as mybir
from concourse import bacc
import concourse.tile as tile
from concourse import bass_utils

NC = 8          # cores
B = 4           # batch
S = 1024        # seq len
D = 1024        # d_model
H = 16          # heads
HD = 64         # head dim
F = 128         # features per core (2 heads * 64)
T = B * S       # 4096 flattened tokens
TC = T // NC    # 512 tokens per core (token shard)
NT = T // 512   # 8 token tiles of 512
NF = D // 128   # 8 feature tiles of 128
EPS = 1e-3
P = 128

FP32 = mybir.dt.float32
BF16 = mybir.dt.bfloat16
AX = mybir.AluOpType
AF = mybir.ActivationFunctionType

CLEAN, BOUNDARY, MASKED = 0, 1, 2
_LAST_NC = None
_LAST_IN_MAPS = None


def _emit(nc, tc, io, cls1, bidx, use_cc=True):
    from contextlib import ExitStack

    n_bnd = max(bidx.values()) + 1 if bidx else 0
    ctx = ExitStack()
    with ctx:
        # ---- pools (bufs is per-tag N-buffering) ----
        wts = ctx.enter_context(tc.tile_pool(name="wts", bufs=3))
        srcp = ctx.enter_context(tc.tile_pool(name="srcp", bufs=2))
        scr = ctx.enter_context(tc.tile_pool(name="scr", bufs=2))
        x3fp = ctx.enter_context(tc.tile_pool(name="x3fp", bufs=8))
        wff = ctx.enter_context(tc.tile_pool(name="wff", bufs=2))
        acts = ctx.enter_context(tc.tile_pool(name="acts", bufs=3))
        epool = ctx.enter_context(tc.tile_pool(name="epool", bufs=2))
        drp = ctx.enter_context(tc.tile_pool(name="drp", bufs=2))
        big = ctx.enter_context(tc.tile_pool(name="big", bufs=1))
        smal = ctx.enter_context(tc.tile_pool(name="smal", bufs=1))
        lns = ctx.enter_context(tc.tile_pool(name="lns", bufs=1))
        bcp = ctx.enter_context(tc.tile_pool(name="bcp", bufs=2))
        ps = ctx.enter_context(tc.tile_pool(name="ps", bufs=4, space="PSUM"))
        pssc = ctx.enter_context(tc.tile_pool(name="pssc", bufs=2, space="PSUM"))
        dram = ctx.enter_context(tc.tile_pool(name="dram", bufs=1, space="DRAM"))

        # ---- constants ----
        ones_col = smal.tile([P, 1], FP32, tag="onesc")
        nc.vector.memset(ones_col[:], 1.0)
        ones_col_bf = smal.tile([P, 1], BF16, tag="onescb")
        nc.vector.memset(ones_col_bf[:], 1.0)
        ones_row = smal.tile([1, P], FP32, tag="onesr")
        nc.vector.memset(ones_row[:], 1.0)
        eps_row = smal.tile([1, 1], FP32, tag="epsr")
        nc.vector.memset(eps_row[:], EPS)
        eps_col = smal.tile([P, 1], FP32, tag="epsc")
        nc.vector.memset(eps_col[:], EPS)
        zero_col = smal.tile([P, 1], FP32, tag="zeroc")
        nc.vector.memset(zero_col[:], 0.0)
        zero_row = smal.tile([1, 1], FP32, tag="zeror")
        nc.vector.memset(zero_row[:], 0.0)


        def load_w(name, dt=BF16):
            w = wts.tile([P, NF * 128], dt, tag="w")
            nc.sync.dma_start(w[:, :].rearrange("p (f m) -> p f m", f=NF),
                              io[name].rearrange("(f p) m -> p f m", p=P))
            return w

        def proj_alloc(n, dts):
            return [acts.tile([P, T], dt, tag="act", name=f"proj_out{wi}")
                    for wi, dt in zip(range(n), dts)]

        def projections(src_ap, w_list, out_dts, has_v, outs=None,
                        chunks=(0, 2, 4, 6, 1, 3, 5, 7), use_sc_psum=False):
            """src_ap: [D, T] dram.  w_list: list of weight sbuf tiles; the
            last one is the V weight if has_v.  Returns per-weight outputs:
            QK-style [P, T] and V token-major [P, 32*128]."""
            if outs is None:
                outs = proj_alloc(len(w_list), out_dts)
            src3 = src_ap.rearrange("(f p) t -> p f t", p=P)
            for j in chunks:
                stile = srcp.tile([P, NF, 512], src_ap.dtype, tag="xsrc")
                nc.sync.dma_start(
                    stile[:, :, :],
                    src3[:, :, j * 512:(j + 1) * 512])
                src = [stile[:, f, :] for f in range(NF)]
                def alloc_pt():
                    if use_sc_psum:
                        return pssc.tile([P, 1024], FP32, tag="sc",
                                         name="projpt")
                    return ps.tile([P, 512], FP32, tag="ps512",
                                   name="projpt")
                nqk = len(w_list) - 1 if has_v else len(w_list)
                for wi in range(nqk):
                    pt = alloc_pt()
                    for f in range(NF):
                        nc.tensor.matmul(
                            pt[:, :512], w_list[wi][:, f * 128:(f + 1) * 128],
                            src[f][:], start=(f == 0), stop=(f == NF - 1))
                    nc.scalar.copy(outs[wi][:, j * 512:(j + 1) * 512],
                                   pt[:, :512])
                if has_v:
                    wv = w_list[-1]
                    vout = outs[-1]
                    for i4 in range(4):
                        i = j * 4 + i4
                        pt = alloc_pt()
                        for f in range(NF):
                            nc.tensor.matmul(
                                pt[:, :128],
                                src[f][:, i4 * 128:(i4 + 1) * 128],
                                wv[:, f * 128:(f + 1) * 128],
                                start=(f == 0), stop=(f == NF - 1))
                        nc.vector.tensor_copy(
                            vout[:, i * 128:(i + 1) * 128], pt[:, :128])
            return outs

        def attn(QT, KT, V, cls, dst_dram, res_sb=None, mid_cb=None,
                 out_sb=None):
            """Head-sharded attention with axis-0 (batch) softmax.
            Writes dst_dram[128*(2b+j):+128, :] = attn_out for tokens of core
            2b+j.  Scores for tile t+1 are emitted before the out-matmuls of
            tile t so the PE never stalls on the softmax.  Fully-masked
            (t,j) tiles contribute exactly 0.25*sum_k V[k] per batch -- done
            as a per-partition bias add instead of 512-col matmuls."""
            for j in range(2):
                ot = [ps.tile([P, 512], FP32, tag="ps512", name=f"ot{b_}")
                      for b_ in range(4)]

                def scores(t):
                    """Emit scores+softmax for tile (t, j); returns W tile
                    or None if fully masked."""
                    tile_cls = cls[t][j]
                    if tile_cls == MASKED:
                        return None
                    # fully-masked columns (q_local < qc) collapse to
                    # W=0.25 exactly; compute softmax only on [qc:512)
                    qc = 128 * (t % 4) if tile_cls == BOUNDARY else 0
                    qw = qc + 128
                    Et = epool.tile([P, 2, 4 * 512], BF16, tag="E")
                    for bp in range(2):
                        pt = [pssc.tile([P, 1024], FP32, tag="sc",
                                        name=f"sc{h_}") for h_ in range(2)]
                        for hh in range(2):
                            for bi in range(2):
                                b = bp * 2 + bi
                                nc.tensor.matmul(
                                    pt[hh][:, bi * 512:(bi + 1) * 512],
                                    KT[64 * hh:64 * (hh + 1),
                                       1024 * b + 128 * t:
                                       1024 * b + 128 * (t + 1)],
                                    QT[64 * hh:64 * (hh + 1),
                                       1024 * b + 512 * j:
                                       1024 * b + 512 * (j + 1)],
                                    start=True, stop=True)
                        if tile_cls == BOUNDARY:
                            sl = bidx[(t, j)]
                            mb = mb_sb[:, sl * 512 + qc:sl * 512 + qw]
                            for hh in range(2):
                                pv = pt[hh][:, :].rearrange(
                                    "p (b q) -> p b q", b=2)[:, :, qc:qw]
                                nc.vector.tensor_tensor(
                                    pv, pv,
                                    mb[:, None, :].broadcast_to(
                                        [P, 2, qw - qc]),
                                    op=AX.mult)
                        for hh in range(2):
                            ev = Et[:, hh,
                                    bp * 1024:(bp + 1) * 1024].rearrange(
                                "p (b q) -> p b q", b=2)
                            pv = pt[hh][:, :].rearrange(
                                "p (b q) -> p b q", b=2)
                            nc.scalar.activation(
                                ev[:, :, qc:], pv[:, :, qc:], AF.Exp,
                                bias=zero_col[:])
                    e4 = Et[:, :, :].rearrange("p h (c q) -> p h c q", c=4)
                    d2 = drp.tile([P, 2, 2, 512], BF16, tag="d2")
                    # batch-pair sums on the (otherwise idle) Pool engine
                    nc.gpsimd.tensor_tensor(
                        d2[:, :, :, qc:], e4[:, :, 0:2, qc:],
                        e4[:, :, 2:4, qc:], op=AX.add)
                    dd = drp.tile([P, 2, 512], BF16, tag="dd")
                    nc.vector.tensor_tensor(
                        dd[:, :, qc:], d2[:, :, 0, qc:], d2[:, :, 1, qc:],
                        op=AX.add)
                    rr = drp.tile([P, 2, 512], BF16, tag="rr")
                    with nc.allow_low_precision(
                            reason="softmax denom ~4.0, bf16 ok"):
                        nc.vector.reciprocal(rr[:, :, qc:], dd[:, :, qc:])
                    # W overwrites E in place (elementwise, same index);
                    # split across DVE and Pool by head to balance engines
                    nc.vector.tensor_tensor(
                        e4[:, 0, :, qc:], e4[:, 0, :, qc:],
                        rr[:, 0, None, qc:].broadcast_to(
                            [P, 4, 512 - qc]),
                        op=AX.mult)
                    nc.gpsimd.tensor_tensor(
                        e4[:, 1, :, qc:], e4[:, 1, :, qc:],
                        rr[:, 1, None, qc:].broadcast_to(
                            [P, 4, 512 - qc]),
                        op=AX.mult)
                    if qc:
                        nc.vector.memset(e4[:, :, :, 0:qc], 0.25)
                    return Et

                masked_ts = [t for t in range(8) if cls[t][j] == MASKED]
                live_ts = [t for t in range(8) if cls[t][j] != MASKED]

                def outmm(t, Wt, first, last):
                    for b in range(4):
                        for hh in range(2):
                            nc.tensor.matmul(
                                ot[b][64 * hh:64 * (hh + 1), :],
                                V[:, 128 * (8 * b + t) + 64 * hh:
                                     128 * (8 * b + t) + 64 * (hh + 1)],
                                Wt[:, hh, b * 512:(b + 1) * 512],
                                start=first, stop=last,
                                tile_position=(0, 64 * hh),
                                skip_group_check=True)

                Wcur = scores(live_ts[0])
                for ti, t in enumerate(live_ts):
                    Wnext = (scores(live_ts[ti + 1])
                             if ti + 1 < len(live_ts) else None)
                    outmm(t, Wcur, ti == 0, ti == len(live_ts) - 1)
                    Wcur = Wnext
                if mid_cb is not None and j == 0:
                    mid_cb()
                vsum = None
                if masked_ts:
                    vsum = drp.tile([P, 4], FP32, tag="vsc")
                    for b in range(4):
                        pv = pssc.tile([P, 1024], FP32, tag="sc",
                                       name=f"vs{b}")
                        for ti, t in enumerate(masked_ts):
                            nc.tensor.matmul(
                                pv[:, 0:1], V[:, 128 * (8 * b + t):
                                              128 * (8 * b + t) + 128],
                                quarter_col[:], start=(ti == 0),
                                stop=(ti == len(masked_ts) - 1))
                        nc.vector.tensor_copy(vsum[:, b:b + 1], pv[:, 0:1])
                for b in range(4):
                    if masked_ts:
                        nc.scalar.activation(ot[b][:], ot[b][:], AF.Identity,
                                             bias=vsum[:, b:b + 1])
                    c_dst = 2 * b + j
                    sl = slice(512 * c_dst, 512 * (c_dst + 1))
                    if out_sb is not None:
                        ob = out_sb[:, sl]
                    else:
                        obt = drp.tile([P, 512], BF16, tag="evac",
                                       name="evac_ob")
                        ob = obt[:, :]
                    if res_sb is not None:
                        nc.vector.tensor_tensor(ob, ot[b][:],
                                                res_sb[:, sl], op=AX.add)
                    else:
                        nc.scalar.copy(ob, ot[b][:])
                    nc.sync.dma_start(
                        dst_dram[128 * c_dst:128 * (c_dst + 1), :], ob)

        def a2a(in_t, out_t):
            if use_cc:
                nc.gpsimd.collective_compute(
                    "AllToAll", AX.bypass, replica_groups=[list(range(NC))],
                    ins=[in_t[:]], outs=[out_t[:]])
            else:
                nc.sync.dma_start(out_t[:], in_t[:])

        def rsqrt_row(var_row):
            """row [1, TC] fp32: returns 1/sqrt(var+eps) via Ln/Exp."""
            nc.scalar.activation(var_row[:], var_row[:], AF.Ln, bias=eps_row[:])
            rr_ = lns.tile([1, TC], FP32, tag="lrr", name="rrrow")
            nc.scalar.activation(rr_[:], var_row[:], AF.Exp, bias=zero_row[:],
                                 scale=-0.5)
            return rr_

        def stats_rows(get_x, nf=NF):
            """x tiles (bf16 or fp32) [P, TC] per f; returns (mu, rr) rows."""
            sp1 = ps.tile([1, TC], FP32, tag="ps512", name="sp1")
            sp2 = ps.tile([1, TC], FP32, tag="ps512", name="sp2")
            for f in range(nf):
                xt = get_x(f)
                oc = ones_col_bf if xt.dtype == BF16 else ones_col
                sq = scr.tile([P, TC], xt.dtype, tag="scrsq")
                nc.vector.tensor_tensor(sq[:], xt[:], xt[:], op=AX.mult)
                nc.tensor.matmul(sp1[:], oc[:], xt[:],
                                 start=(f == 0), stop=(f == nf - 1))
                nc.tensor.matmul(sp2[:], oc[:], sq[:],
                                 start=(f == 0), stop=(f == nf - 1))
            mu_ = lns.tile([1, TC], FP32, tag="lmu", name="murow")
            nc.vector.tensor_scalar_mul(mu_[:], sp1[:], 1.0 / D)
            mub_ = lns.tile([1, TC], BF16, tag="lmub", name="murowb")
            nc.vector.tensor_copy(mub_[:], mu_[:])
            mu2_ = lns.tile([1, TC], FP32, tag="lmu2")
            nc.vector.tensor_tensor(mu2_[:], mu_[:], mu_[:], op=AX.mult)
            var_ = lns.tile([1, TC], FP32, tag="lvar")
            nc.vector.scalar_tensor_tensor(var_[:], sp2[:], 1.0 / D, mu2_[:],
                                           op0=AX.mult, op1=AX.subtract)
            return mu_, mub_, rsqrt_row(var_)

        def bcast(row):
            """[1, TC] -> SBUF [P, TC] broadcast (via PE + copy; SBUF so
            consumers can pair it with a PSUM operand)."""
            pb = ps.tile([P, TC], FP32, tag="ps512", name="bcast")
            nc.tensor.matmul(pb[:], ones_row[:], row[:], start=True, stop=True)
            sb = bcp.tile([P, TC], FP32, tag="bcsb", name="bcast_sb")
            nc.scalar.copy(sb[:], pb[:])
            return sb

        # ================= MHA1 (head-sharded, no comm) =================
        wq1 = load_w("wq1s")
        wk1 = load_w("wk1s")
        wv1 = load_w("wv1s")
        mb_sb = None
        qkv1 = proj_alloc(3, [BF16, BF16, BF16])
        # even token chunks first: attn1 j=0 (q<512 per batch) only needs
        # these, so its softmax pipeline starts while odd chunks project
        projections(io["xdT"], [wq1, wk1, wv1], [BF16, BF16, BF16],
                    has_v=True, outs=qkv1, chunks=(0, 2, 4, 6))
        QT1, KT1, V1 = qkv1
        xdres = big.tile([P, T], BF16, tag="xdres")
        nc.sync.dma_start(xdres[:], io["xd_res"][:])
        if n_bnd:
            mb_sb = smal.tile([P, n_bnd * 512], BF16, tag="mb")
            nc.sync.dma_start(mb_sb[:], io["mbndbar"][:])
        ident = smal.tile([P, P], BF16, tag="ident")
        nc.sync.dma_start(ident[:], io["ident128"][:])
        quarter_col = smal.tile([P, 1], BF16, tag="qcol")
        nc.vector.memset(quarter_col[:], 0.25)
        a2a1_in = dram.tile([D, TC], BF16)
        a2a1_out = dram.tile([D, TC], BF16)

        def _odds():
            projections(io["xdT"], [wq1, wk1, wv1], [BF16, BF16, BF16],
                        has_v=True, outs=qkv1, chunks=(1, 3, 5, 7),
                        use_sc_psum=True)
        attn(QT1, KT1, V1, cls1, a2a1_in, res_sb=xdres, mid_cb=_odds)
        a2a(a2a1_in, a2a1_out)

        # K2/V2 projections (input-only deps) fill the A2A#1 window
        wk2 = load_w("wk2s")
        wv2 = load_w("wv2s")
        KT2, V2 = projections(io["xeT"], [wk2, wv2], [BF16, BF16],
                              has_v=True)

        # full Wq2 (pre-scaled 1/sqrt(hd)) + negated col-sums
        wq2f = smal.tile([P, NF * D], BF16, tag="wq2f")
        nc.sync.dma_start(wq2f[:, :].rearrange("p (f m) -> p f m", f=NF),
                          io["wq2f"].rearrange("(f p) m -> p f m", p=P))
        q2negs = smal.tile([1, D], BF16, tag="q2negs")
        nc.sync.dma_start(q2negs[:], io["wq2negs"][:])
        w1negs = smal.tile([1, D], BF16, tag="w1negs")
        nc.sync.dma_start(w1negs[:], io["w1negs"][:])

        # ============ post-A2A#1: x1 (token-sharded), stats, Q2 ============
        x1bf = big.tile([P, NF, TC], BF16, tag="a1ld", name="x1bf")
        nc.sync.dma_start(x1bf[:, :, :],
                          a2a1_out.rearrange("(f p) t -> p f t", p=P))
        mu1, mu1b, rr1 = stats_rows(lambda f: x1bf[:, f, :])

        a2a2_in = dram.tile([D, TC], BF16)
        a2a2_out = dram.tile([D, TC], BF16)
        rrb1 = bcast(rr1)
        for hq in range(NF):
            pq = pssc.tile([P, 1024], FP32, tag="sc", name=f"pq{hq}")
            for f in range(NF):
                nc.tensor.matmul(pq[:, :TC],
                                 wq2f[:, f * D + hq * 128:
                                      f * D + (hq + 1) * 128],
                                 x1bf[:, f, :], start=(f == 0), stop=False)
            nc.tensor.matmul(pq[:, :TC],
                             q2negs[0:1, hq * 128:(hq + 1) * 128],
                             mu1b[:], start=False, stop=True)
            q2b = scr.tile([P, TC], BF16, tag="scrb")
            nc.vector.tensor_tensor(q2b[:], pq[:, :TC], rrb1[:], op=AX.mult)
            nc.sync.dma_start(a2a2_in[128 * hq:128 * (hq + 1), :], q2b[:])
        a2a(a2a2_in, a2a2_out)

        # A2A#2 window fillers: a = (x1-mu)*r (Pool) and ha = W1^T a (PE)
        mub1 = bcast(mu1)
        a_bf = big.tile([P, NF, TC], BF16, tag="abf")
        for f in range(NF):
            tt = scr.tile([P, TC], FP32, tag="scr")
            nc.gpsimd.tensor_tensor(tt[:], x1bf[:, f, :], mub1[:],
                                    op=AX.subtract)
            nc.gpsimd.tensor_tensor(a_bf[:, f, :], tt[:], rrb1[:],
                                    op=AX.mult)

        ha_bf = big.tile([P, NF, TC], BF16, tag="hbf", name="ha_bf")
        for hq in range(NF):
            w1t = wff.tile([P, NF * 128], BF16, tag="wt")
            nc.sync.dma_start(
                w1t[:, :].rearrange("p (f m) -> p f m", f=NF),
                io["w1"][:, hq * 128:(hq + 1) * 128]
                .rearrange("(f p) m -> p f m", p=P))
            pa = ps.tile([P, TC], FP32, tag="ps512", name=f"pa{hq}")
            for f in range(NF):
                nc.tensor.matmul(pa[:], w1t[:, f * 128:(f + 1) * 128],
                                 a_bf[:, f, :],
                                 start=(f == 0), stop=(f == NF - 1))
            nc.scalar.copy(ha_bf[:, hq, :], pa[:])

        QT2 = acts.tile([P, T], BF16, tag="act", name="QT2")
        qt2v = QT2[:, :].rearrange("p (c t) -> p c t", c=NC)
        a2a2v = a2a2_out.rearrange("(c p) t -> p c t", p=P)
        for c_ in (0, 2, 4, 6, 1, 3, 5, 7):
            nc.sync.dma_start(qt2v[:, c_, :], a2a2v[:, c_, :])
        # ================= MHA2 (head-sharded) =================
        cls_clean = [[CLEAN] * 2 for _ in range(8)]
        a2a3_in = dram.tile([D, TC], BF16)
        a2a3_out = dram.tile([D, TC], BF16)
        attn(QT2, KT2, V2, cls_clean, a2a3_in)
        a2a(a2a3_in, a2a3_out)

        # ============ tail: x2, LN2-fixup, FFN, LN3 (token-sharded) ========
        a3ld = big.tile([P, NF, TC], BF16, tag="a1ld", name="a3ld")
        nc.sync.dma_start(a3ld[:, :, :],
                          a2a3_out.rearrange("(f p) t -> p f t", p=P))
        x2bf = big.tile([P, NF, TC], BF16, tag="x2bf")
        for f in range(NF):
            eng = nc.vector if f % 2 == 0 else nc.gpsimd
            eng.tensor_tensor(x2bf[:, f, :], a3ld[:, f, :],
                              a_bf[:, f, :], op=AX.add)
        mu2, mu2b, rr2 = stats_rows(lambda f: x2bf[:, f, :])
        rrb2 = bcast(rr2)

        h_bf = big.tile([P, NF, TC], BF16, tag="hbf2")
        for hq in range(NF):
            w1t = wff.tile([P, NF * 128], BF16, tag="wt")
            nc.sync.dma_start(
                w1t[:, :].rearrange("p (f m) -> p f m", f=NF),
                io["w1"][:, hq * 128:(hq + 1) * 128]
                .rearrange("(f p) m -> p f m", p=P))
            ph = pssc.tile([P, 1024], FP32, tag="sc", name=f"ph{hq}")
            for f in range(NF):
                nc.tensor.matmul(ph[:, :TC], w1t[:, f * 128:(f + 1) * 128],
                                 a3ld[:, f, :], start=(f == 0), stop=False)
            nc.tensor.matmul(ph[:, :TC], ident[:], ha_bf[:, hq, :],
                             start=False, stop=False)
            nc.tensor.matmul(ph[:, :TC],
                             w1negs[0:1, hq * 128:(hq + 1) * 128],
                             mu2b[:], start=False, stop=True)
            nc.vector.tensor_tensor(h_bf[:, hq, :], ph[:, :TC], rrb2[:],
                                    op=AX.mult)

        # c = (x2-mu2)*r2 fp32 (residual for x3); fills PE-wait gaps
        mub2 = bcast(mu2)
        c_t = big.tile([P, NF, TC], BF16, tag="c_t")
        for f in range(NF):
            tt = scr.tile([P, TC], FP32, tag="scr")
            nc.gpsimd.tensor_tensor(tt[:], x2bf[:, f, :], mub2[:],
                                    op=AX.subtract)
            nc.gpsimd.tensor_tensor(c_t[:, f, :], tt[:], rrb2[:],
                                    op=AX.mult)

        sp31 = ps.tile([1, TC], FP32, tag="ps512", name="sp31")
        sp32 = ps.tile([1, TC], FP32, tag="ps512", name="sp32")
        x3f = []
        for oq in range(NF):
            w2t = wff.tile([P, NF * 128], BF16, tag="wt")
            nc.sync.dma_start(
                w2t[:, :].rearrange("p (f m) -> p f m", f=NF),
                io["w2"][:, oq * 128:(oq + 1) * 128]
                .rearrange("(f p) m -> p f m", p=P))
            px = pssc.tile([P, 1024], FP32, tag="sc", name=f"px{oq}")
            for f in range(NF):
                nc.tensor.matmul(px[:, :TC], w2t[:, f * 128:(f + 1) * 128],
                                 h_bf[:, f, :],
                                 start=(f == 0), stop=(f == NF - 1))
            x3 = x3fp.tile([P, TC], FP32, tag="x3f")
            nc.vector.scalar_tensor_tensor(
                x3[:], px[:, :TC], 1.0, c_t[:, oq, :],
                op0=AX.mult, op1=AX.add)
            sq = scr.tile([P, TC], BF16, tag="scrsq", name="sq3")
            nc.gpsimd.tensor_tensor(sq[:], x3[:], x3[:], op=AX.mult)
            nc.tensor.matmul(sp31[:], ones_col[:], x3[:],
                             start=(oq == 0), stop=(oq == NF - 1))
            nc.tensor.matmul(sp32[:], ones_col_bf[:], sq[:],
                             start=(oq == 0), stop=(oq == NF - 1))
            x3f.append(x3)
        mu3 = lns.tile([1, TC], FP32, tag="lmu", name="mu3row")
        nc.vector.tensor_scalar_mul(mu3[:], sp31[:], 1.0 / D)
        mu23 = lns.tile([1, TC], FP32, tag="lmu2", name="mu23")
        nc.vector.tensor_tensor(mu23[:], mu3[:], mu3[:], op=AX.mult)
        var3 = lns.tile([1, TC], FP32, tag="lvar", name="var3")
        nc.vector.scalar_tensor_tensor(var3[:], sp32[:], 1.0 / D, mu23[:],
                                       op0=AX.mult, op1=AX.subtract)
        rr3 = rsqrt_row(var3)
        rrb3 = bcast(rr3)
        mub3 = bcast(mu3)
        out3 = io["out"].rearrange("(f p) t -> p f t", p=P)
        for f in range(NF):
            eng = nc.vector if f % 2 == 0 else nc.gpsimd
            tt = scr.tile([P, TC], FP32, tag="scr")
            eng.tensor_tensor(tt[:], x3f[f][:], mub3[:], op=AX.subtract)
            eng.tensor_tensor(x3f[f][:], tt[:], rrb3[:], op=AX.mult)
            nc.sync.dma_start(out3[:, f, :], x3f[f][:])


def _build(cls1, bidx, use_cc=True, num_devices=NC):
    nc = bacc.Bacc("TRN2", target_bir_lowering=False, debug=False,
                   num_devices=num_devices)
    n_bnd = max(bidx.values()) + 1 if bidx else 0
    io = {}

    def inp(name, shape, dt=FP32):
        io[name] = nc.dram_tensor(name, shape, dt, kind="ExternalInput").ap()

    inp("xdT", [D, T], BF16); inp("xeT", [D, T], BF16)
    inp("xd_res", [F, T], BF16)
    inp("wq1s", [D, F], BF16); inp("wk1s", [D, F], BF16)
    inp("wv1s", [D, F], BF16)
    inp("wk2s", [D, F], BF16); inp("wv2s", [D, F], BF16)
    inp("wq2f", [D, D], BF16)
    inp("wq2negs", [1, D], BF16); inp("w1negs", [1, D], BF16)
    inp("w1", [D, D], BF16); inp("w2", [D, D], BF16)
    inp("ident128", [P, P], BF16)
    if n_bnd:
        inp("mbndbar", [128, n_bnd * 512], BF16)
    io["out"] = nc.dram_tensor("out", [D, TC], FP32, kind="ExternalOutput").ap()

    with tile.TileContext(nc) as tc:
        _emit(nc, tc, io, cls1, bidx, use_cc=use_cc)
    nc.compile()
    return nc


def _classify(mT):
    cls = [[CLEAN] * 2 for _ in range(8)]
    bidx = {}
    for t in range(8):
        for j in range(2):
            sub = mT[128 * t:128 * (t + 1), 512 * j:512 * (j + 1)]
            if sub.max() == 0:
                cls[t][j] = CLEAN
            elif sub.min() == 1:
                cls[t][j] = MASKED
            else:
                cls[t][j] = BOUNDARY
                bidx[(t, j)] = len(bidx)
    return cls, bidx


def kernel(**inputs):
    f32 = np.float32
    bf16 = ml_dtypes.bfloat16
    dec = np.asarray(inputs["dec_input"], f32)
    en = np.asarray(inputs["en_input"], f32)
    lam = np.asarray(inputs["look_ahead_mask"], f32)
    msk2 = np.asarray(inputs["mask"], f32)

    assert np.all(msk2 == 0.0), "cross-attention mask expected to be zero"
    assert np.all((lam == 0.0) | (lam == 1.0)), "mask must be binary"
    assert np.all(lam == lam[0:1]), "mask must be batch-uniform"
    for nm in ("bq1", "bk1", "bv1", "bq2", "bk2", "bv2", "bf1", "bf2",
               "be1", "be2", "be3"):
        assert np.all(np.asarray(inputs[nm]) == 0.0), f"{nm} expected zero"
    for nm in ("g1", "g2", "g3"):
        assert np.all(np.asarray(inputs[nm]) == 1.0), f"{nm} expected one"

    mT = np.ascontiguousarray(lam[0, 0].T).astype(f32)  # [k, q]
    cls1, bidx = _classify(mT)
    n_bnd = len(bidx)

    xdT = np.ascontiguousarray(dec.reshape(T, D).T)
    xeT = np.ascontiguousarray(en.reshape(T, D).T)

    mbndbar = np.zeros((128, max(n_bnd, 1) * 512), bf16)
    for (t, j), sl in bidx.items():
        sub = mT[128 * t:128 * (t + 1), 512 * j:512 * (j + 1)]
        mbndbar[:, sl * 512:(sl + 1) * 512] = 1.0 - sub

    Wq1 = np.asarray(inputs["Wq1"], f32); Wk1 = np.asarray(inputs["Wk1"], f32)
    Wv1 = np.asarray(inputs["Wv1"], f32)
    Wq2 = np.asarray(inputs["Wq2"], f32); Wk2 = np.asarray(inputs["Wk2"], f32)
    Wv2 = np.asarray(inputs["Wv2"], f32)
    W1 = np.asarray(inputs["W1"], f32); W2 = np.asarray(inputs["W2"], f32)
    scale = f32(1.0) / np.sqrt(f32(HD))

    xdT_bf = xdT.astype(bf16)
    xeT_bf = xeT.astype(bf16)
    wq2f_bf = (Wq2 * scale).astype(bf16)
    wq2negs_full = (-(wq2f_bf.astype(f32).sum(axis=0))
                    .reshape(1, D).astype(bf16))
    w1_bf = W1.astype(bf16)
    w1negs = (-(w1_bf.astype(f32).sum(axis=0))
              .reshape(1, D).astype(bf16))
    w2_bf = W2.astype(bf16)

    in_maps = []
    for c in range(NC):
        sl = slice(F * c, F * (c + 1))
        m = {
            "xdT": xdT_bf, "xeT": xeT_bf,
            "xd_res": np.ascontiguousarray(xdT_bf[sl]),
            "wq1s": np.ascontiguousarray(Wq1[:, sl] * scale).astype(bf16),
            "wk1s": np.ascontiguousarray(Wk1[:, sl]).astype(bf16),
            "wv1s": np.ascontiguousarray(Wv1[:, sl]).astype(bf16),
            "wk2s": np.ascontiguousarray(Wk2[:, sl]).astype(bf16),
            "wv2s": np.ascontiguousarray(Wv2[:, sl]).astype(bf16),
            "wq2f": wq2f_bf,
            "wq2negs": wq2negs_full, "w1negs": w1negs,
            "w1": w1_bf, "w2": w2_bf,
            "ident128": np.eye(P, dtype=bf16),
        }
        if n_bnd:
            m["mbndbar"] = mbndbar
        in_maps.append(m)

    global _LAST_NC, _LAST_IN_MAPS
    nc = _build(cls1, bidx)
    _LAST_NC, _LAST_IN_MAPS = nc, in_maps
    res = bass_utils.run_bass_kernel_spmd(nc, in_maps, core_ids=list(range(NC)))

    outT = np.empty((D, T), f32)
    for c in range(NC):
        outT[:, TC * c:TC * (c + 1)] = res.results[c]["out"]
    return np.ascontiguousarray(outT.T).reshape(B, S, D).astype(np.float32)
